# revision 9
# baseline (speedup 1.0000x reference)
"""Trainium2 Bass kernel for Graphormer multi-head attention.

Reference computation (per batch b of 16, nh=12 heads, N=512 tokens, H=768):
    q = x @ Wq + bq; k = x @ Wk + bk; v = x @ Wv + bv      (x nodes-first (N,B,H))
    scores = q k^T / sqrt(64) + attention_bias[b]
    attn = softmax(scores, axis=-1)   (key_padding_mask all-False)
    out = (attn @ v) @ Wo + bo

Sharding: batch dim (16) split across 8 NeuronCores, 2 batches per core.
On-device everything is kept feature-major ("transposed") so no transposes
are ever needed:
    xT (H,N) -> QT/KT (H,N) via weight-stationary matmuls,
    V (N,H) token-major via x-stationary matmuls,
    ST = scores^T (m,n) = KT^T-slices @ QT  per head,
    PT = exp(ST + biasT) with bias pre-transposed on host (fp16),
    rowsums via ones-vector matmuls, attn@v as V-stationary matmuls
    producing out^T (d,n), normalized by 1/rowsum broadcast via a PE
    outer-product, final y^T = Wo^T-form matmul.
All matmuls run in float32r (~1.9e-4 rel err, 4x the fp32 matmul rate).
"""

import numpy as np

try:
    import concourse  # noqa: F401
except ImportError:
    import sys

    sys.path.insert(0, "/opt/trn_rl_repo")

import concourse.bass as bass  # noqa: E402
import concourse.mybir as mybir  # noqa: E402
import concourse.tile as tile  # noqa: E402
from concourse import bacc  # noqa: E402
from concourse.bass_utils import run_bass_kernel_spmd  # noqa: E402

NCORES = 8
B, NH, N, H, HD = 16, 12, 512, 768, 64
BL = B // NCORES  # batches per core = 2
NPAIR = NH // 2  # head pairs = 6
NMC = N // 128  # token m-chunks = 4
NJC = H // 128  # feature chunks = 6

F32 = mybir.dt.float32
F32R = mybir.dt.float32r
F16 = mybir.dt.float16
AF = mybir.ActivationFunctionType

_COMPILED = {"nc": None}
LAST_RESULTS = None  # BassKernelResults of the most recent kernel() call


def _emit(nc, tc, ctx):
    """Emit the per-core kernel body (SPMD; each core handles BL batches)."""
    xT_d = nc.dram_tensor("xT", [BL, H, N], F32R, kind="ExternalInput")
    biasT_d = nc.dram_tensor("biasT", [BL, NH, N, N], F16, kind="ExternalInput")
    wq_d = nc.dram_tensor("Wq", [H, H], F32R, kind="ExternalInput")
    wk_d = nc.dram_tensor("Wk", [H, H], F32R, kind="ExternalInput")
    wv_d = nc.dram_tensor("Wv", [H, H], F32R, kind="ExternalInput")
    wo_d = nc.dram_tensor("Wo", [H, H], F32R, kind="ExternalInput")
    pbias_d = nc.dram_tensor("pbias", [128, 18], F32, kind="ExternalInput")
    ones_d = nc.dram_tensor("ones_c", [128, 64], F32R, kind="ExternalInput")
    yT_d = nc.dram_tensor("yT", [BL, H, N], F32, kind="ExternalOutput")

    const = ctx.enter_context(tc.tile_pool(name="const", bufs=1))
    wpool = ctx.enter_context(tc.tile_pool(name="wpool", bufs=1))
    xpool = ctx.enter_context(tc.tile_pool(name="xpool", bufs=1))
    qkv = ctx.enter_context(tc.tile_pool(name="qkv", bufs=1))
    ppool = ctx.enter_context(tc.tile_pool(name="ppool", bufs=2))
    bpool = ctx.enter_context(tc.tile_pool(name="bpool", bufs=4))
    spool = ctx.enter_context(tc.tile_pool(name="spool", bufs=2))
    ypool = ctx.enter_context(tc.tile_pool(name="ypool", bufs=2))
    ps_sc = ctx.enter_context(tc.tile_pool(name="ps_sc", bufs=2, space="PSUM"))
    ps_av = ctx.enter_context(tc.tile_pool(name="ps_av", bufs=1, space="PSUM"))
    ps_sm = ctx.enter_context(tc.tile_pool(name="ps_sm", bufs=1, space="PSUM"))
    ps_pj = ctx.enter_context(tc.tile_pool(name="ps_pj", bufs=2, space="PSUM"))

    # weights, resident for the whole kernel
    wq_sb = wpool.tile([128, NJC, NJC, 128], F32R, tag="wq")
    wk_sb = wpool.tile([128, NJC, NJC, 128], F32R, tag="wk")
    wo_sb = wpool.tile([128, NJC, NJC, 128], F32R, tag="wo")
    for w_sb, w_d in ((wq_sb, wq_d), (wk_sb, wk_d), (wo_sb, wo_d)):
        nc.sync.dma_start(
            out=w_sb,
            in_=w_d.ap().rearrange("(ic p) (jc q) -> p ic jc q", p=128, q=128),
        )
    wv_sb = wpool.tile([128, NJC, H], F32R, tag="wv")
    nc.sync.dma_start(out=wv_sb, in_=wv_d.ap().rearrange("(ic p) j -> p ic j", p=128))
    pbias_sb = const.tile([128, 18], F32, tag="pbias")
    nc.sync.dma_start(out=pbias_sb, in_=pbias_d.ap())
    ones_sb = const.tile([128, 64], F32R, tag="ones")
    nc.sync.dma_start(out=ones_sb, in_=ones_d.ap())

    for b in range(BL):
        xT_sb = xpool.tile([128, NJC, N], F32R, tag="xT")
        nc.sync.dma_start(
            out=xT_sb, in_=xT_d.ap()[b].rearrange("(ic p) n -> p ic n", p=128)
        )

        # ---- projections ----
        qT_sb = qkv.tile([128, NJC, N], F32R, tag="qT")
        kT_sb = qkv.tile([128, NJC, N], F32R, tag="kT")
        for w_sb, dst, col0, scale in ((wq_sb, qT_sb, 0, 0.125), (wk_sb, kT_sb, 6, 1.0)):
            for jc in range(NJC):
                pj = ps_pj.tile([128, 512], F32, tag="pj")
                for ic in range(NJC):
                    nc.tensor.matmul(
                        pj,
                        w_sb[:, ic, jc, :],
                        xT_sb[:, ic, :],
                        start=(ic == 0),
                        stop=(ic == NJC - 1),
                    )
                nc.scalar.activation(
                    out=dst[:, jc, :],
                    in_=pj,
                    func=AF.Identity,
                    bias=pbias_sb[:, col0 + jc : col0 + jc + 1],
                    scale=scale,
                )
        v_sb = qkv.tile([128, NMC, H], F32R, tag="v")
        for mc in range(NMC):
            for fc in range(2):  # feature halves of 384
                pj = ps_pj.tile([128, 512], F32, tag="pj")
                pjv = pj[:, 0:384]
                for ic in range(NJC):
                    nc.tensor.matmul(
                        pjv,
                        xT_sb[:, ic, mc * 128 : (mc + 1) * 128],
                        wv_sb[:, ic, fc * 384 : (fc + 1) * 384],
                        start=(ic == 0),
                        stop=(ic == NJC - 1),
                    )
                nc.scalar.activation(
                    out=v_sb[:, mc, fc * 384 : (fc + 1) * 384],
                    in_=pjv,
                    func=AF.Copy,
                )

        # ---- attention, software-pipelined over head pairs ----
        # stage 1 (pair ph):   scoresT = kT.T-slices @ qT  (+biasT, exp) -> PT
        # stage 2 (pair ph-1): attn@v + dup-rowsums -> 1/sums -> normalize
        outcT_sb = qkv.tile([128, NJC, N], F32R, tag="oT")
        pT_tiles = {}

        def scores_stage(ph):
            pT_sb = ppool.tile([128, NMC, 1024], F32R, tag="pT")
            pT_tiles[ph] = pT_sb
            for mc in range(NMC):
                bias_sb = bpool.tile([128, 1024], F16, tag="bias")
                nc.sync.dma_start(
                    out=bias_sb,
                    in_=biasT_d.ap()[b, 2 * ph : 2 * ph + 2, mc * 128 : (mc + 1) * 128, :]
                    .rearrange("h m n -> m h n"),
                )
                sc = ps_sc.tile([128, 1024], F32, tag="sc")
                for hp in range(2):
                    sl = slice(hp * 64, hp * 64 + 64)
                    nc.tensor.matmul(
                        sc[:, hp * 512 : (hp + 1) * 512],
                        kT_sb[sl, ph, mc * 128 : (mc + 1) * 128],
                        qT_sb[sl, ph, :],
                        start=True,
                        stop=True,
                        tile_position=(hp * 64, 0),
                    )
                nc.vector.tensor_add(sc, sc, bias_sb)
                nc.scalar.activation(out=pT_sb[:, mc, :], in_=sc, func=AF.Exp)

        def reduce_stage(ph):
            pT_sb = pT_tiles.pop(ph)
            for hp in range(2):
                hg = 2 * ph + hp
                av = ps_av.tile([64, 512], F32, tag="av")
                sm = ps_sm.tile([64, 512], F32, tag="sm")
                for mc in range(NMC):
                    nc.tensor.matmul(
                        av,
                        v_sb[:, mc, hg * 64 : hg * 64 + 64],
                        pT_sb[:, mc, hp * 512 : (hp + 1) * 512],
                        start=(mc == 0),
                        stop=(mc == NMC - 1),
                    )
                for mc in range(NMC):
                    # ones lhsT with M=64 -> 64 duplicated rowsum rows; the
                    # duplication IS the partition broadcast for normalize.
                    nc.tensor.matmul(
                        sm,
                        ones_sb[:, 0:64],
                        pT_sb[:, mc, hp * 512 : (hp + 1) * 512],
                        start=(mc == 0),
                        stop=(mc == NMC - 1),
                    )
                inv_sb = spool.tile([64, 512], F32, tag="inv")
                nc.vector.reciprocal(inv_sb, sm)
                if hp == 0:
                    nc.vector.tensor_mul(outcT_sb[0:64, ph, :], av, inv_sb)
                else:
                    # DVE lanes cannot shift partitions; bounce through SBUF DMA
                    tmp_sb = spool.tile([64, 512], F32R, tag="tmp")
                    nc.vector.tensor_mul(tmp_sb, av, inv_sb)
                    nc.sync.dma_start(out=outcT_sb[64:128, ph, :], in_=tmp_sb)

        for ph in range(NPAIR + 1):
            if ph < NPAIR:
                scores_stage(ph)
            if ph >= 1:
                reduce_stage(ph - 1)

        # ---- output projection ----
        for jc in range(NJC):
            pj = ps_pj.tile([128, 512], F32, tag="pj")
            for ic in range(NJC):
                nc.tensor.matmul(
                    pj,
                    wo_sb[:, ic, jc, :],
                    outcT_sb[:, ic, :],
                    start=(ic == 0),
                    stop=(ic == NJC - 1),
                )
            y_sb = ypool.tile([128, 512], F32, tag="y")
            nc.scalar.activation(
                out=y_sb,
                in_=pj,
                func=AF.Identity,
                bias=pbias_sb[:, 12 + jc : 12 + jc + 1],
            )
            nc.sync.dma_start(
                out=yT_d.ap()[b, jc * 128 : (jc + 1) * 128, :], in_=y_sb
            )


def _build():
    if _COMPILED["nc"] is None:
        from contextlib import ExitStack

        nc = bacc.Bacc("TRN2", target_bir_lowering=False, debug=False)
        with tile.TileContext(nc) as tc, ExitStack() as ctx:
            _emit(nc, tc, ctx)
        nc.compile()
        _COMPILED["nc"] = nc
    return _COMPILED["nc"]


def kernel(
    x, attention_bias, key_padding_mask, Wq, bq, Wk, bk, Wv, bv, Wo, bo, **_unused
):
    global LAST_RESULTS
    x = np.asarray(x, dtype=np.float32)
    attention_bias = np.asarray(attention_bias, dtype=np.float32)
    key_padding_mask = np.asarray(key_padding_mask)
    Wq, bq, Wk, bk = (np.asarray(a, dtype=np.float32) for a in (Wq, bq, Wk, bk))
    Wv, bv, Wo, bo = (np.asarray(a, dtype=np.float32) for a in (Wv, bv, Wo, bo))

    nc = _build()

    # projection biases: columns 0-5 = bq/8 (the 1/sqrt(hd) scale is folded into
    # the Q psum->sbuf copy), 6-11 = bk, 12-17 = bo + bv @ Wo (the V bias
    # commutes through softmax-weighted averaging into the output projection).
    bo_eff = bo + bv @ Wo
    pb = np.zeros((128, 18), np.float32)
    pb[:, 0:6] = (bq * 0.125).reshape(6, 128).T
    pb[:, 6:12] = bk.reshape(6, 128).T
    pb[:, 12:18] = bo_eff.reshape(6, 128).T

    ones_c = np.ones((128, 64), np.float32)
    in_maps = []
    for c in range(NCORES):
        bsl = slice(c * BL, (c + 1) * BL)
        xT = np.ascontiguousarray(x[:, bsl, :].transpose(1, 2, 0))
        biasT = attention_bias[bsl].transpose(0, 1, 3, 2)
        mask = key_padding_mask[bsl]
        if mask.any():
            biasT = biasT.copy()
            for bl in range(BL):
                biasT[bl][:, mask[bl], :] = -30000.0
        in_maps.append(
            {
                "xT": xT,
                "biasT": np.ascontiguousarray(biasT.astype(np.float16)),
                "Wq": Wq,
                "Wk": Wk,
                "Wv": Wv,
                "Wo": Wo,
                "pbias": pb,
                "ones_c": ones_c,
            }
        )

    res = run_bass_kernel_spmd(nc, in_maps, list(range(NCORES)))
    LAST_RESULTS = res

    out = np.empty((N, B, H), np.float32)
    for c in range(NCORES):
        yT = res.results[c]["yT"]  # (BL, H, N)
        out[:, c * BL : (c + 1) * BL, :] = yT.transpose(2, 0, 1)
    return out
